# revision 14
# baseline (speedup 1.0000x reference)
"""TRN2 Bass/Tile kernel for nn_CAttention (b=4, c=384, hw=128x128, 8 heads).

Math (per batch b):
  qkv = qkv_w @ x            [1152, n]   (1x1 conv, n = h*w = 16384)
  q, k l2-normalized along n; attn = softmax((q @ k^T) * temp) per head [48,48]
  out = proj_w @ concat_h(attn_h @ v_h)  [384, n]

Sharding: 8 cores = (batch 0..3) x (head-half 0..1). Each core:
  - computes q,k,v for its 4 heads ([576, n] slice of qkv rows),
  - streams over n in blocks, accumulating raw scores S_raw = q_raw @ k_raw^T
    and per-row sum-of-squares (normalization applied to S post-hoc:
    S = S_raw * outer(1/||q_c||, 1/||k_d||) * temp),
  - softmax -> attn; M2.T = blockdiag(attn).T @ Wp_local.T  [192, 384]
    (attn@v and the projection fuse into a single matmul out = M2 @ v),
  - writes a partial projection [384, n]; host sums the two head-half
    partials per batch.

PE layouts (out = lhsT.T @ rhs, contraction on partitions):
  qkv:    lhsT = qkv_w_sel.T [384, 576] chunks, rhs = x [384, n] chunks.
  S_raw:  q,k transposed to [n, c] via PE transposes per [128,128] block,
          two heads ("pair") packed per matmul [128, 96].
  M2 @ v: lhsT = M2.T [192, 384] pair-chunks, rhs = v [96x2, n] SBUF-resident.

Data tiles are fp16 (PE full rate, fp32 PSUM accumulation); error vs the
fp32 reference is ~1e-4.
"""

import numpy as np

import concourse.bass as bass
import concourse.mybir as mybir
import concourse.tile as tile
from concourse import bacc, bass_utils
from concourse.masks import make_identity

FP32 = mybir.dt.float32
FP16 = mybir.dt.float16
AF = mybir.ActivationFunctionType
ALU = mybir.AluOpType
AX = mybir.AxisListType

# problem dims
B = 4
C = 384
H = W = 128
N = H * W          # 16384
HEADS = 8
CH = 48            # per-head channels
HLOC = 4           # heads per core
CLOC = HLOC * CH   # 192

# dtypes for the fast paths
DT = FP16          # qk blocks / transposes / S_raw / v / M2
NPDT = np.float16

# tiling
NB = 2048          # n-block for the streaming phase
NBLK = N // NB     # 8
NCHUNK = NB // 128 # 16 transpose chunks / block
N512 = NB // 512   # 4 psum chunks / block
MT_SIZES = (128, 128, 128, 128, 64)  # qkv output row tiles (576 rows)


def build_nc():
    nc = bacc.Bacc("TRN2", target_bir_lowering=False, debug=False, num_devices=8)

    x_in = nc.dram_tensor("x_in", [C, N], DT, kind="ExternalInput").ap()
    wqkvT = nc.dram_tensor("wqkvT", [C, 3 * CLOC], DT, kind="ExternalInput").ap()
    wprojT = nc.dram_tensor("wprojT", [CLOC, C], FP32, kind="ExternalInput").ap()
    tempq = nc.dram_tensor("tempq", [1, 2 * CLOC], FP32, kind="ExternalInput").ap()
    # additive softmax mask: 0 on the two diagonal 48-blocks, -1e30 elsewhere
    bdmask = nc.dram_tensor("bdmask", [96, 96], FP32, kind="ExternalInput").ap()
    outp = nc.dram_tensor("outp", [C, N], FP32, kind="ExternalOutput").ap()

    x_v = x_in.rearrange("(t p) n -> t p n", p=128)    # [3, 128, N]
    w_v = wqkvT.rearrange("(t p) m -> t p m", p=128)   # [3, 128, 576]
    wp_v = wprojT.rearrange("(s p) m -> s p m", p=96)  # [2, 96, 384]
    o_v = outp.rearrange("(t p) n -> t p n", p=128)    # [3, 128, N]

    with tile.TileContext(nc) as tc:
        with (
            tc.tile_pool(name="const", bufs=1) as constp,
            tc.tile_pool(name="vres", bufs=1) as vpool,
            tc.tile_pool(name="xin", bufs=2) as xpool,
            tc.tile_pool(name="qk", bufs=2) as qkpool,
            tc.tile_pool(name="qkt", bufs=2) as qktpool,
            tc.tile_pool(name="sq", bufs=2) as sqpool,
            tc.tile_pool(name="small", bufs=1) as smallp,
            tc.tile_pool(name="ostage", bufs=2) as opool,
            tc.tile_pool(name="mm_psum", bufs=4, space="PSUM") as mmps,
            tc.tile_pool(name="tr_psum", bufs=2, space="PSUM") as trps,
            tc.tile_pool(name="s_psum", bufs=1, space="PSUM") as sps,
        ):
            # ---- constants / weights ----
            ident = constp.tile([128, 128], DT, tag="ident")
            make_identity(nc, ident[:])
            identf = constp.tile([128, 128], FP32, tag="identf")
            make_identity(nc, identf[:])

            w_sb = constp.tile([128, 3, 3 * CLOC], DT, tag="wqkv")
            for kc in range(3):
                nc.sync.dma_start(w_sb[:, kc, :], w_v[kc])
            wp_sb = constp.tile([96, 2, C], FP32, tag="wproj")
            for cc in range(2):
                nc.sync.dma_start(wp_sb[:, cc, :], wp_v[cc])
            tq_sb = constp.tile([1, 2 * CLOC], FP32, tag="tq")
            nc.sync.dma_start(tq_sb[:], tempq[:])
            mask_sb = constp.tile([96, 96], FP32, tag="mask")
            nc.sync.dma_start(mask_sb[:], bdmask[:])

            # ---- persistent accumulators ----
            v_sb = vpool.tile([96, 2, N], DT, tag="v")            # v, pair-major
            nsq = NBLK * N512  # one sumsq partial column per 512-chunk
            ssq_parts = smallp.tile([128, 3 * nsq], FP32, tag="ssqp")
            s_acc = smallp.tile([96, 2 * 96], FP32, tag="sacc")   # S_raw, per pair

            # ================= phase 1: stream over n =================
            for blk in range(NBLK):
                n0 = blk * NB
                x_t = xpool.tile([128, 3, NB], DT, tag="x")
                for ct in range(3):
                    nc.sync.dma_start(x_t[:, ct, :], x_v[ct, :, n0 : n0 + NB])

                qk_t = qkpool.tile([128, 3, NB], DT, tag="qk")
                for mt in range(5):
                    msz = MT_SIZES[mt]
                    r0 = mt * 128
                    for n2 in range(N512):
                        ps = mmps.tile([128, 512], FP32, tag="mm")
                        for kc in range(3):
                            nc.tensor.matmul(
                                ps[:msz, :],
                                lhsT=w_sb[:, kc, r0 : r0 + msz],
                                rhs=x_t[:, kc, n2 * 512 : (n2 + 1) * 512],
                                start=(kc == 0),
                                stop=(kc == 2),
                            )
                        c0 = n0 + n2 * 512
                        if mt < 3:  # q,k rows -> block tile (cast to fp16)
                            nc.vector.tensor_copy(
                                qk_t[:, mt, n2 * 512 : (n2 + 1) * 512], ps[:, :]
                            )
                        elif mt == 3:  # v rows 384:512
                            nc.vector.tensor_copy(
                                v_sb[0:96, 0, c0 : c0 + 512], ps[0:96, :]
                            )
                            nc.vector.tensor_copy(
                                v_sb[0:32, 1, c0 : c0 + 512], ps[96:128, :]
                            )
                        else:  # v rows 512:576 (32-partition aligned windows)
                            nc.vector.tensor_copy(
                                v_sb[32:64, 1, c0 : c0 + 512], ps[0:32, :]
                            )
                            nc.vector.tensor_copy(
                                v_sb[64:96, 1, c0 : c0 + 512], ps[32:64, :]
                            )

                # sum-of-squares partials (ACT engine; accumulates per row).
                # one 512-chunk per op so each waits on a single producer copy
                for ct in range(3):
                    for n2 in range(N512):
                        sqs = sqpool.tile([128, 512], DT, tag="sq")
                        col = ct * nsq + blk * N512 + n2
                        nc.scalar.activation(
                            sqs[:, :],
                            qk_t[:, ct, n2 * 512 : (n2 + 1) * 512],
                            AF.Square,
                            accum_out=ssq_parts[:, col : col + 1],
                        )

                # transpose q,k chunks -> [n, c] layout
                qkt_t = qktpool.tile([128, NCHUNK, 2 * CLOC], DT, tag="qkt")
                for ct in range(3):
                    for ncc in range(NCHUNK):
                        tp = trps.tile([128, 128], DT, tag="tr")
                        nc.tensor.transpose(
                            tp[:, :],
                            qk_t[:, ct, ncc * 128 : (ncc + 1) * 128],
                            ident[:, :],
                        )
                        nc.vector.tensor_copy(
                            qkt_t[:, ncc, ct * 128 : (ct + 1) * 128], tp[:, :]
                        )

                # raw scores, accumulated over the block's chunks in PSUM
                sp0 = sps.tile([96, 96], FP32, tag="s0")
                sp1 = sps.tile([96, 96], FP32, tag="s1")
                for ncc in range(NCHUNK):
                    for pr, sp in enumerate((sp0, sp1)):
                        nc.tensor.matmul(
                            sp[:, :],
                            lhsT=qkt_t[:, ncc, pr * 96 : (pr + 1) * 96],
                            rhs=qkt_t[:, ncc, 192 + pr * 96 : 192 + (pr + 1) * 96],
                            start=(ncc == 0),
                            stop=(ncc == NCHUNK - 1),
                        )
                for pr, sp in enumerate((sp0, sp1)):
                    dst = s_acc[:, pr * 96 : (pr + 1) * 96]
                    if blk == 0:
                        nc.vector.tensor_copy(dst, sp[:, :])
                    else:
                        nc.vector.tensor_add(dst, dst, sp[:, :])

            # ================= phase 2: attention finalize =================
            ssq = smallp.tile([128, 3], FP32, tag="ssq")
            for ct in range(3):
                nc.vector.tensor_reduce(
                    ssq[:, ct : ct + 1],
                    ssq_parts[:, ct * nsq : (ct + 1) * nsq],
                    axis=AX.X,
                    op=ALU.add,
                )
            rec = smallp.tile([128, 3], FP32, tag="rec")
            nc.vector.reciprocal(rec[:, :], ssq[:, :])
            rnorm = smallp.tile([128, 3], FP32, tag="rnorm")
            nc.scalar.activation(rnorm[:, :], rec[:, :], AF.Sqrt)

            # move 1/||.|| into a single row [1, 384] via PE transposes
            scale_row = smallp.tile([1, 2 * CLOC], FP32, tag="srow")
            for ct in range(3):
                tp = trps.tile([1, 128], FP32, tag="tr")
                nc.tensor.transpose(tp[:, :], rnorm[:, ct : ct + 1], identf[:, :])
                nc.vector.tensor_copy(
                    scale_row[0:1, ct * 128 : (ct + 1) * 128], tp[:, :]
                )
            # fold temperature into the q-side scales
            nc.vector.tensor_mul(scale_row[:, :], scale_row[:, :], tq_sb[:, :])

            # scale S_raw, softmax per head, assemble blockdiag attn
            attn_bd = smallp.tile([96, 2, CLOC], FP32, tag="abd")
            nc.vector.memset(attn_bd[:], 0.0)
            for pr in range(2):
                op = sps.tile([96, 96], FP32, tag="s0")
                nc.tensor.matmul(
                    op[:, :],
                    lhsT=scale_row[0:1, pr * 96 : (pr + 1) * 96],
                    rhs=scale_row[0:1, 192 + pr * 96 : 192 + (pr + 1) * 96],
                    start=True,
                    stop=True,
                )
                spr = s_acc[:, pr * 96 : (pr + 1) * 96]
                nc.vector.tensor_mul(spr, spr, op[:, :])
                # mask off cross-head entries, then softmax over the 96 cols
                nc.vector.tensor_add(spr, spr, mask_sb[:, :])
                nmax = smallp.tile([96, 1], FP32, tag="nmax")
                nc.vector.tensor_reduce(
                    nmax[:, :], spr, axis=AX.X, op=ALU.max, negate=True
                )
                esum = smallp.tile([96, 1], FP32, tag="esum")
                dst = attn_bd[:, pr, pr * 96 : (pr + 1) * 96]
                nc.scalar.activation(
                    dst, spr, AF.Exp, bias=nmax[:, :], accum_out=esum[:, :]
                )
                rcp = smallp.tile([96, 1], FP32, tag="rcp")
                nc.vector.reciprocal(rcp[:, :], esum[:, :])
                nc.vector.tensor_scalar_mul(dst, dst, rcp[:, :])

            # M2.T = blockdiag(attn).T @ Wp_local.T   [192, 384], pair chunks
            m2t = smallp.tile([96, 2, C], DT, tag="m2t")
            for pr in range(2):
                ps = mmps.tile([128, 512], FP32, tag="mm")
                for cc in range(2):
                    nc.tensor.matmul(
                        ps[0:96, 0:C],
                        lhsT=attn_bd[:, cc, pr * 96 : (pr + 1) * 96],
                        rhs=wp_sb[:, cc, :],
                        start=(cc == 0),
                        stop=(cc == 1),
                    )
                nc.vector.tensor_copy(m2t[:, pr, :], ps[0:96, 0:C])

            # ================= phase 3: out = M2 @ v =================
            for n5 in range(N // 512):
                ot = opool.tile([128, 3, 512], FP32, tag="ost")
                for mt in range(3):
                    ps = mmps.tile([128, 512], FP32, tag="mm")
                    for pr in range(2):
                        nc.tensor.matmul(
                            ps[:, :],
                            lhsT=m2t[:, pr, mt * 128 : (mt + 1) * 128],
                            rhs=v_sb[:, pr, n5 * 512 : (n5 + 1) * 512],
                            start=(pr == 0),
                            stop=(pr == 1),
                        )
                    nc.vector.tensor_copy(ot[:, mt, :], ps[:, :])
                    nc.sync.dma_start(
                        o_v[mt, :, n5 * 512 : (n5 + 1) * 512], ot[:, mt, :]
                    )
    nc.compile()
    return nc


_NC = None


def _get_nc():
    global _NC
    if _NC is None:
        _NC = build_nc()
    return _NC


def make_in_maps(x, qkv_w, proj_w, temperature):
    x2 = np.ascontiguousarray(np.asarray(x).reshape(B, C, N).astype(NPDT))
    qkv_w = np.asarray(qkv_w, np.float32)
    proj_w = np.asarray(proj_w, np.float32)
    tvals = np.asarray(temperature, np.float32).reshape(HEADS)
    in_maps = []
    for core in range(8):
        bi, hg = core // 2, core % 2
        heads = list(range(hg * HLOC, (hg + 1) * HLOC))
        rows = [t * C + hh * CH + j for t in range(3) for hh in heads for j in range(CH)]
        wqkvT = np.ascontiguousarray(qkv_w[rows, :].T.astype(NPDT))  # [384, 576]
        cols = [hh * CH + j for hh in heads for j in range(CH)]
        wprojT = np.ascontiguousarray(proj_w[:, cols].T)             # [192, 384]
        tq = np.ones((1, 2 * CLOC), np.float32)
        for j, hh in enumerate(heads):
            tq[0, j * CH : (j + 1) * CH] = tvals[hh]
        mask = np.full((96, 96), -1e30, np.float32)
        mask[:48, :48] = 0.0
        mask[48:, 48:] = 0.0
        in_maps.append(
            {"x_in": x2[bi], "wqkvT": wqkvT, "wprojT": wprojT, "tempq": tq,
             "bdmask": mask}
        )
    return in_maps


def kernel(x, qkv_w, proj_w, temperature, **run_kwargs):
    nc = _get_nc()
    in_maps = make_in_maps(x, qkv_w, proj_w, temperature)
    res = bass_utils.run_bass_kernel_spmd(
        nc, in_maps, core_ids=list(range(8)), **run_kwargs
    )
    outs = [r["outp"] for r in res.results]
    out = np.empty((B, C, N), np.float32)
    for bi in range(B):
        out[bi] = outs[2 * bi] + outs[2 * bi + 1]
    out = out.reshape(B, C, H, W)
    kernel.last_results = res
    return out


# revision 30
# speedup vs baseline: 1.3659x; 1.3659x over previous
"""TRN2 Bass/Tile kernel for nn_CAttention (b=4, c=384, hw=128x128, 8 heads).

Math (per batch b, x flattened to [384, n], n = 16384):
  qkv = qkv_w @ x; q,k l2-normalized along n;
  attn = softmax((q_n @ k_n^T) * temp) per head [48,48]
  out = proj_w @ concat_h(attn_h @ v_h)

Sharding: 8 cores = (batch 0..3) x (head-half 0..1); host sums the two
head-half partial projections per batch.

Per-core pipeline (4 local heads = 2 "pairs" of 2 heads):
  phase 1 (stream n in 2048-blocks, x parked SBUF-resident as fp16):
    q,k produced directly TRANSPOSED ([n, 384] layout) via x-stationary
    matmuls.  Raw scores S = q_raw @ k_raw^T accumulate in PSUM per pair;
    the same matmul's moving operand carries [k_pair | q_pair] so its
    columns 96:192 give the q Gram block (diag = sum-of-squares for the
    l2 norm); a second small matmul gives the k Gram.  Normalization is
    applied post-hoc: S <- S * outer(1/||q_c||, 1/||k_d||) * temp.
  phase 2: Gram diagonals -> 1/sqrt -> row vectors (tiny identity
    matmuls), outer-product scaling, masked softmax per pair ([96,96]
    with -1e30 off-block additive mask), blockdiag attn assembly.
    M2.T = blockdiag(attn).T @ Wp_local.T;  M3.T = Wv_local.T @ M2.T --
    attn@v and the projection fold into a single [384,384] matrix M3.
  phase 3: out_partial = M3 @ x from resident x, streamed to DRAM.

Data tiles fp16 (PE full rate, fp32 PSUM accumulation); end-to-end error
vs the fp32 reference ~4e-4.

Hardware notes: tensor_tensor_reduce reading PSUM crashes the exec unit;
gram diagonals use tensor_mul + tensor_reduce instead.  GPSIMD C-axis
reduce is element-serial (~1 elem/ns) - never use it for bulk data.
"""

import numpy as np

import concourse.bass as bass
import concourse.mybir as mybir
import concourse.tile as tile
from concourse import bacc, bass_utils
from concourse.masks import make_identity

FP32 = mybir.dt.float32
FP16 = mybir.dt.float16
AF = mybir.ActivationFunctionType
ALU = mybir.AluOpType
AX = mybir.AxisListType

# problem dims
B = 4
C = 384
H = W = 128
N = H * W          # 16384
HEADS = 8
CH = 48            # per-head channels
HLOC = 4           # heads per core
CLOC = HLOC * CH   # 192

DT = FP16
NPDT = np.float16

# tiling
NB = 2048          # n-block for the streaming phase
NBLK = N // NB     # 8
NCHUNK = NB // 128 # 16 chunks / block
N512 = NB // 512   # 4


def build_nc():
    nc = bacc.Bacc("TRN2", target_bir_lowering=False, debug=False, num_devices=8)

    # wqkvT columns: [kA(96) | qA(96) | kB(96) | qB(96)]
    x_in = nc.dram_tensor("x_in", [C, N], DT, kind="ExternalInput").ap()
    wqkvT = nc.dram_tensor("wqkvT", [C, 2 * CLOC], DT, kind="ExternalInput").ap()
    wprojT = nc.dram_tensor("wprojT", [CLOC, C], FP32, kind="ExternalInput").ap()
    wv = nc.dram_tensor("wv", [CLOC, C], DT, kind="ExternalInput").ap()
    tempc = nc.dram_tensor("tempc", [96, 4], FP32, kind="ExternalInput").ap()
    bdmask = nc.dram_tensor("bdmask", [96, 96], FP32, kind="ExternalInput").ap()
    outp = nc.dram_tensor("outp", [C, N], FP32, kind="ExternalOutput").ap()

    x_v = x_in.rearrange("(t p) n -> t p n", p=128)    # [3, 128, N]
    w_v = wqkvT.rearrange("(t p) m -> t p m", p=128)   # [3, 128, 384]
    wp_v = wprojT.rearrange("(s p) m -> s p m", p=96)  # [2, 96, 384]
    wv_v = wv.rearrange("(s p) m -> s p m", p=96)      # [2, 96, 384]
    o_v = outp.rearrange("(t p) n -> t p n", p=128)    # [3, 128, N]

    with tile.TileContext(nc) as tc:
        with (
            tc.tile_pool(name="const", bufs=1) as constp,
            tc.tile_pool(name="xres", bufs=1) as xpool,
            tc.tile_pool(name="qkt", bufs=2) as qktpool,
            tc.tile_pool(name="small", bufs=1) as smallp,
            tc.tile_pool(name="scr", bufs=2) as scrp,
            tc.tile_pool(name="ostage", bufs=3) as opool,
            tc.tile_pool(name="mm_psum", bufs=4, space="PSUM") as mmps,
            tc.tile_pool(name="s_psum", bufs=1, space="PSUM") as sps,
        ):
            # ---- constants / weights ----
            ident96 = constp.tile([96, 96], FP32, tag="ident96")
            make_identity(nc, ident96[:])

            w_sb = constp.tile([128, 3, 2 * CLOC], DT, tag="wqkv")
            for kc in range(3):
                nc.sync.dma_start(w_sb[:, kc, :], w_v[kc])
            wp_sb = constp.tile([96, 2, C], FP32, tag="wproj")
            for cc in range(2):
                nc.sync.dma_start(wp_sb[:, cc, :], wp_v[cc])
            wv_sb = constp.tile([96, 2, C], DT, tag="wv")
            for cc in range(2):
                nc.sync.dma_start(wv_sb[:, cc, :], wv_v[cc])
            tc_sb = constp.tile([96, 4], FP32, tag="tempc")
            nc.sync.dma_start(tc_sb[:], tempc[:])
            mask_sb = constp.tile([96, 96], FP32, tag="mask")
            nc.sync.dma_start(mask_sb[:], bdmask[:])

            # ---- persistent state ----
            x_res = xpool.tile([128, 3, N], DT, tag="xres")       # resident input
            s_acc = smallp.tile([96, 2 * 96], FP32, tag="sacc")   # S_raw per pair
            # gram-diag partials: col = (pr*2 + {0:q,1:k}) * NBLK + blk
            ssq_parts = smallp.tile([96, 4 * NBLK], FP32, tag="ssqp")

            # ================= phase 1: stream over n =================
            for blk in range(NBLK):
                n0 = blk * NB
                for ct in range(3):
                    nc.sync.dma_start(
                        x_res[:, ct, n0 : n0 + NB], x_v[ct, :, n0 : n0 + NB]
                    )

                # q,k transposed: [n-chunk, 384] = x_chunk.T @ Wqk.T
                qkt_t = qktpool.tile([128, NCHUNK, 2 * CLOC], DT, tag="qkt")
                for ch in range(NCHUNK):
                    ps = mmps.tile([128, 512], FP32, tag="mm")
                    for kc in range(3):
                        nc.tensor.matmul(
                            ps[:, 0 : 2 * CLOC],
                            lhsT=x_res[:, kc, n0 + ch * 128 : n0 + (ch + 1) * 128],
                            rhs=w_sb[:, kc, :],
                            start=(kc == 0),
                            stop=(kc == 2),
                        )
                    if ch % 2 == 0:
                        nc.scalar.copy(qkt_t[:, ch, :], ps[:, 0 : 2 * CLOC])
                    else:
                        nc.vector.tensor_copy(qkt_t[:, ch, :], ps[:, 0 : 2 * CLOC])

                # scores + gram blocks, accumulated over chunks in PSUM
                sg = [sps.tile([96, 192], FP32, tag=f"s{pr}", name=f"sg{pr}")
                      for pr in range(2)]
                gk = [sps.tile([96, 96], FP32, tag=f"g{pr}", name=f"gk{pr}")
                      for pr in range(2)]
                for ch in range(NCHUNK):
                    for pr in range(2):
                        o = pr * 192
                        # stationary q_pair; moving [k_pair | q_pair]
                        nc.tensor.matmul(
                            sg[pr][:, :],
                            lhsT=qkt_t[:, ch, o + 96 : o + 192],
                            rhs=qkt_t[:, ch, o : o + 192],
                            start=(ch == 0),
                            stop=(ch == NCHUNK - 1),
                        )
                        nc.tensor.matmul(
                            gk[pr][:, :],
                            lhsT=qkt_t[:, ch, o : o + 96],
                            rhs=qkt_t[:, ch, o : o + 96],
                            start=(ch == 0),
                            stop=(ch == NCHUNK - 1),
                        )
                for pr in range(2):
                    dst = s_acc[:, pr * 96 : (pr + 1) * 96]
                    if blk == 0:
                        nc.vector.tensor_copy(dst, sg[pr][:, 0:96])
                    else:
                        nc.vector.tensor_add(dst, dst, sg[pr][:, 0:96])
                    # gram diagonals via identity-mask + row-reduce
                    cq = (pr * 2) * NBLK + blk
                    ck = (pr * 2 + 1) * NBLK + blk
                    scr = scrp.tile([96, 96], FP32, tag="scr")
                    nc.vector.tensor_mul(scr[:, :], sg[pr][:, 96:192], ident96[:, :])
                    nc.vector.tensor_reduce(
                        ssq_parts[:, cq : cq + 1], scr[:, :],
                        axis=AX.X, op=ALU.add,
                    )
                    scr2 = scrp.tile([96, 96], FP32, tag="scr")
                    nc.vector.tensor_mul(scr2[:, :], gk[pr][:, :], ident96[:, :])
                    nc.vector.tensor_reduce(
                        ssq_parts[:, ck : ck + 1], scr2[:, :],
                        axis=AX.X, op=ALU.add,
                    )

            # ================= phase 2: attention finalize =================
            # cols of ssqt: 0=qA 1=kA 2=qB 3=kB
            ssqt = smallp.tile([96, 4], FP32, tag="ssqt")
            for i in range(4):
                nc.vector.tensor_reduce(
                    ssqt[:, i : i + 1],
                    ssq_parts[:, i * NBLK : (i + 1) * NBLK],
                    axis=AX.X,
                    op=ALU.add,
                )
            rec = smallp.tile([96, 4], FP32, tag="rec")
            nc.vector.reciprocal(rec[:, :], ssqt[:, :])
            rn = smallp.tile([96, 4], FP32, tag="rn")
            nc.scalar.activation(rn[:, :], rec[:, :], AF.Sqrt)
            # fold temperature into the q-side scales (tempc: ones in k cols)
            nc.vector.tensor_mul(rn[:, :], rn[:, :], tc_sb[:, :])

            # per-column -> row vectors: plain K=96 matmul against identity
            srow = smallp.tile([1, 4 * 96], FP32, tag="srow")
            for i in range(4):
                tp = sps.tile([1, 96], FP32, tag="g0", name="tp")
                nc.tensor.matmul(
                    tp[:, :], lhsT=rn[:, i : i + 1], rhs=ident96[:, :],
                    start=True, stop=True,
                )
                nc.vector.tensor_copy(srow[0:1, i * 96 : (i + 1) * 96], tp[:, :])

            # scale S, masked softmax per pair, assemble blockdiag attn
            attn_bd = smallp.tile([96, 2, CLOC], FP32, tag="abd")
            nc.vector.memset(attn_bd[:], 0.0)
            for pr in range(2):
                op = sps.tile([96, 96], FP32, tag="g0")
                nc.tensor.matmul(
                    op[:, :],
                    lhsT=srow[0:1, (2 * pr) * 96 : (2 * pr + 1) * 96],
                    rhs=srow[0:1, (2 * pr + 1) * 96 : (2 * pr + 2) * 96],
                    start=True,
                    stop=True,
                )
                spr = s_acc[:, pr * 96 : (pr + 1) * 96]
                nc.vector.tensor_mul(spr, spr, op[:, :])
                nc.vector.tensor_add(spr, spr, mask_sb[:, :])
                nmax = smallp.tile([96, 1], FP32, tag="nmax")
                nc.vector.tensor_reduce(
                    nmax[:, :], spr, axis=AX.X, op=ALU.max, negate=True
                )
                esum = smallp.tile([96, 1], FP32, tag="esum")
                dst = attn_bd[:, pr, pr * 96 : (pr + 1) * 96]
                nc.scalar.activation(
                    dst, spr, AF.Exp, bias=nmax[:, :], accum_out=esum[:, :]
                )
                rcp = smallp.tile([96, 1], FP32, tag="rcp")
                nc.vector.reciprocal(rcp[:, :], esum[:, :])
                nc.vector.tensor_scalar_mul(dst, dst, rcp[:, :])

            # M2.T = blockdiag(attn).T @ Wp_local.T   [192, 384], pair chunks
            m2t = smallp.tile([96, 2, C], DT, tag="m2t")
            for pr in range(2):
                ps = mmps.tile([128, 512], FP32, tag="mm")
                for cc in range(2):
                    nc.tensor.matmul(
                        ps[0:96, 0:C],
                        lhsT=attn_bd[:, cc, pr * 96 : (pr + 1) * 96],
                        rhs=wp_sb[:, cc, :],
                        start=(cc == 0),
                        stop=(cc == 1),
                    )
                nc.vector.tensor_copy(m2t[:, pr, :], ps[0:96, 0:C])

            # M3.T = Wv_local.T @ M2.T  [384, 384] = ((proj @ attn_bd) @ Wv).T
            m3t = smallp.tile([128, 3, C], DT, tag="m3t")
            for mt in range(3):
                ps = mmps.tile([128, 512], FP32, tag="mm")
                for cc in range(2):
                    nc.tensor.matmul(
                        ps[:, 0:C],
                        lhsT=wv_sb[:, cc, mt * 128 : (mt + 1) * 128],
                        rhs=m2t[:, cc, :],
                        start=(cc == 0),
                        stop=(cc == 1),
                    )
                nc.vector.tensor_copy(m3t[:, mt, :], ps[:, 0:C])

            # ============= phase 3: out = M3 @ x (resident) =============
            for blk in range(NBLK):
                n0 = blk * NB
                for mt in range(3):
                    pss = [mmps.tile([128, 512], FP32, tag="mm", name=f"po{i}")
                           for i in range(N512)]
                    for kc in range(3):
                        for n5 in range(N512):
                            nc.tensor.matmul(
                                pss[n5][:, :],
                                lhsT=m3t[:, kc, mt * 128 : (mt + 1) * 128],
                                rhs=x_res[:, kc, n0 + n5 * 512 : n0 + (n5 + 1) * 512],
                                start=(kc == 0),
                                stop=(kc == 2),
                            )
                    for n5 in range(N512):
                        ot = opool.tile([128, 512], FP32, tag="ost")
                        if n5 % 3 == 0:
                            nc.scalar.copy(ot[:, :], pss[n5][:, :])
                        else:
                            nc.vector.tensor_copy(ot[:, :], pss[n5][:, :])
                        nc.sync.dma_start(
                            o_v[mt, :, n0 + n5 * 512 : n0 + (n5 + 1) * 512],
                            ot[:, :],
                        )
    nc.compile()
    return nc


_NC = None


def _get_nc():
    global _NC
    if _NC is None:
        _NC = build_nc()
    return _NC


def make_in_maps(x, qkv_w, proj_w, temperature):
    x2 = np.ascontiguousarray(np.asarray(x).reshape(B, C, N).astype(NPDT))
    qkv_w = np.asarray(qkv_w, np.float32)
    proj_w = np.asarray(proj_w, np.float32)
    tvals = np.asarray(temperature, np.float32).reshape(HEADS)
    in_maps = []
    for core in range(8):
        bi, hg = core // 2, core % 2
        heads = list(range(hg * HLOC, (hg + 1) * HLOC))
        # qkv rows in wqkvT column order [kA | qA | kB | qB]
        rows = []
        for pr in range(2):
            ph = heads[2 * pr : 2 * pr + 2]
            rows += [C + hh * CH + j for hh in ph for j in range(CH)]      # k
            rows += [hh * CH + j for hh in ph for j in range(CH)]          # q
        wqkvT = np.ascontiguousarray(qkv_w[rows, :].T.astype(NPDT))  # [384, 384]
        vrows = [2 * C + hh * CH + j for hh in heads for j in range(CH)]
        wvm = np.ascontiguousarray(qkv_w[vrows, :].astype(NPDT))     # [192, 384]
        cols = [hh * CH + j for hh in heads for j in range(CH)]
        wprojT = np.ascontiguousarray(proj_w[:, cols].T)             # [192, 384]
        # tempc cols: 0=qA 1=kA 2=qB 3=kB; temperature applies to q side
        tcol = np.ones((96, 4), np.float32)
        for pr in range(2):
            ph = heads[2 * pr : 2 * pr + 2]
            tcol[0:48, 2 * pr] = tvals[ph[0]]
            tcol[48:96, 2 * pr] = tvals[ph[1]]
        mask = np.full((96, 96), -1e30, np.float32)
        mask[:48, :48] = 0.0
        mask[48:, 48:] = 0.0
        in_maps.append(
            {"x_in": x2[bi], "wqkvT": wqkvT, "wprojT": wprojT, "wv": wvm,
             "tempc": tcol, "bdmask": mask}
        )
    return in_maps


def kernel(x, qkv_w, proj_w, temperature, **run_kwargs):
    nc = _get_nc()
    in_maps = make_in_maps(x, qkv_w, proj_w, temperature)
    res = bass_utils.run_bass_kernel_spmd(
        nc, in_maps, core_ids=list(range(8)), **run_kwargs
    )
    outs = [r["outp"] for r in res.results]
    out = np.empty((B, C, N), np.float32)
    for bi in range(B):
        out[bi] = outs[2 * bi] + outs[2 * bi + 1]
    out = out.reshape(B, C, H, W)
    kernel.last_results = res
    return out


# revision 31
# speedup vs baseline: 1.4094x; 1.0319x over previous
"""TRN2 Bass/Tile kernel for nn_CAttention (b=4, c=384, hw=128x128, 8 heads).

Math (per batch b, x flattened to [384, n], n = 16384):
  qkv = qkv_w @ x; q,k l2-normalized along n;
  attn = softmax((q_n @ k_n^T) * temp) per head [48,48]
  out = proj_w @ concat_h(attn_h @ v_h)

Sharding: 8 cores = (batch 0..3) x (head-half 0..1); host sums the two
head-half partial projections per batch.

Per-core pipeline (4 local heads = 2 "pairs" of 2 heads):
  phase 1 (stream n in 2048-blocks, x parked SBUF-resident as fp16):
    q,k produced directly TRANSPOSED ([n, 384] layout) via x-stationary
    matmuls.  Raw scores S = q_raw @ k_raw^T accumulate in PSUM per pair;
    the same matmul's moving operand carries [k_pair | q_pair] so its
    columns 96:192 give the q Gram block (diag = sum-of-squares for the
    l2 norm); a second small matmul gives the k Gram.  Normalization is
    applied post-hoc: S <- S * outer(1/||q_c||, 1/||k_d||) * temp.
  phase 2: Gram diagonals -> 1/sqrt -> row vectors (tiny identity
    matmuls), outer-product scaling, masked softmax per pair ([96,96]
    with -1e30 off-block additive mask), blockdiag attn assembly.
    M2.T = blockdiag(attn).T @ Wp_local.T;  M3.T = Wv_local.T @ M2.T --
    attn@v and the projection fold into a single [384,384] matrix M3.
  phase 3: out_partial = M3 @ x from resident x, streamed to DRAM.

Data tiles fp16 (PE full rate, fp32 PSUM accumulation); end-to-end error
vs the fp32 reference ~4e-4.

Hardware notes: tensor_tensor_reduce reading PSUM crashes the exec unit;
gram diagonals use tensor_mul + tensor_reduce instead.  GPSIMD C-axis
reduce is element-serial (~1 elem/ns) - never use it for bulk data.
"""

import numpy as np

import concourse.bass as bass
import concourse.mybir as mybir
import concourse.tile as tile
from concourse import bacc, bass_utils
from concourse.masks import make_identity

FP32 = mybir.dt.float32
FP16 = mybir.dt.float16
AF = mybir.ActivationFunctionType
ALU = mybir.AluOpType
AX = mybir.AxisListType

# problem dims
B = 4
C = 384
H = W = 128
N = H * W          # 16384
HEADS = 8
CH = 48            # per-head channels
HLOC = 4           # heads per core
CLOC = HLOC * CH   # 192

DT = FP16
NPDT = np.float16

# tiling
NB = 2048          # n-block for the streaming phase
NBLK = N // NB     # 8
NCHUNK = NB // 128 # 16 chunks / block
N512 = NB // 512   # 4


def build_nc():
    nc = bacc.Bacc("TRN2", target_bir_lowering=False, debug=False, num_devices=8)

    # wqkvT columns: [kA(96) | qA(96) | kB(96) | qB(96)]
    x_in = nc.dram_tensor("x_in", [C, N], DT, kind="ExternalInput").ap()
    wqkvT = nc.dram_tensor("wqkvT", [C, 2 * CLOC], DT, kind="ExternalInput").ap()
    wprojT = nc.dram_tensor("wprojT", [CLOC, C], FP32, kind="ExternalInput").ap()
    wv = nc.dram_tensor("wv", [CLOC, C], DT, kind="ExternalInput").ap()
    tempc = nc.dram_tensor("tempc", [96, 4], FP32, kind="ExternalInput").ap()
    bdmask = nc.dram_tensor("bdmask", [96, 96], FP32, kind="ExternalInput").ap()
    outp = nc.dram_tensor("outp", [C, N // 2], FP32, kind="ExternalOutput").ap()

    x_v = x_in.rearrange("(t p) n -> t p n", p=128)    # [3, 128, N]
    w_v = wqkvT.rearrange("(t p) m -> t p m", p=128)   # [3, 128, 384]
    wp_v = wprojT.rearrange("(s p) m -> s p m", p=96)  # [2, 96, 384]
    wv_v = wv.rearrange("(s p) m -> s p m", p=96)      # [2, 96, 384]
    o_v = outp.rearrange("(t p) n -> t p n", p=128)    # [3, 128, N/2]

    with tile.TileContext(nc) as tc:
        with (
            tc.tile_pool(name="const", bufs=1) as constp,
            tc.tile_pool(name="xres", bufs=1) as xpool,
            tc.tile_pool(name="qkt", bufs=2) as qktpool,
            tc.tile_pool(name="small", bufs=1) as smallp,
            tc.tile_pool(name="scr", bufs=2) as scrp,
            tc.tile_pool(name="ostage", bufs=3) as opool,
            tc.tile_pool(name="mm_psum", bufs=4, space="PSUM") as mmps,
            tc.tile_pool(name="s_psum", bufs=1, space="PSUM") as sps,
            tc.tile_pool(name="dram", bufs=1, space="DRAM") as dramp,
        ):
            # ---- constants / weights ----
            ident96 = constp.tile([96, 96], FP32, tag="ident96")
            make_identity(nc, ident96[:])

            w_sb = constp.tile([128, 3, 2 * CLOC], DT, tag="wqkv")
            for kc in range(3):
                nc.sync.dma_start(w_sb[:, kc, :], w_v[kc])
            wp_sb = constp.tile([96, 2, C], FP32, tag="wproj")
            for cc in range(2):
                nc.sync.dma_start(wp_sb[:, cc, :], wp_v[cc])
            wv_sb = constp.tile([96, 2, C], DT, tag="wv")
            for cc in range(2):
                nc.sync.dma_start(wv_sb[:, cc, :], wv_v[cc])
            tc_sb = constp.tile([96, 4], FP32, tag="tempc")
            nc.sync.dma_start(tc_sb[:], tempc[:])
            mask_sb = constp.tile([96, 96], FP32, tag="mask")
            nc.sync.dma_start(mask_sb[:], bdmask[:])

            # ---- persistent state ----
            x_res = xpool.tile([128, 3, N], DT, tag="xres")       # resident input
            s_acc = smallp.tile([96, 2 * 96], FP32, tag="sacc")   # S_raw per pair
            # gram-diag partials: col = (pr*2 + {0:q,1:k}) * NBLK + blk
            ssq_parts = smallp.tile([96, 4 * NBLK], FP32, tag="ssqp")

            # ================= phase 1: stream over n =================
            for blk in range(NBLK):
                n0 = blk * NB
                for ct in range(3):
                    nc.sync.dma_start(
                        x_res[:, ct, n0 : n0 + NB], x_v[ct, :, n0 : n0 + NB]
                    )

                # q,k transposed: [n-chunk, 384] = x_chunk.T @ Wqk.T
                qkt_t = qktpool.tile([128, NCHUNK, 2 * CLOC], DT, tag="qkt")
                for ch in range(NCHUNK):
                    ps = mmps.tile([128, 512], FP32, tag="mm")
                    for kc in range(3):
                        nc.tensor.matmul(
                            ps[:, 0 : 2 * CLOC],
                            lhsT=x_res[:, kc, n0 + ch * 128 : n0 + (ch + 1) * 128],
                            rhs=w_sb[:, kc, :],
                            start=(kc == 0),
                            stop=(kc == 2),
                        )
                    if ch % 2 == 0:
                        nc.scalar.copy(qkt_t[:, ch, :], ps[:, 0 : 2 * CLOC])
                    else:
                        nc.vector.tensor_copy(qkt_t[:, ch, :], ps[:, 0 : 2 * CLOC])

                # scores + gram blocks, accumulated over chunks in PSUM
                sg = [sps.tile([96, 192], FP32, tag=f"s{pr}", name=f"sg{pr}")
                      for pr in range(2)]
                gk = [sps.tile([96, 96], FP32, tag=f"g{pr}", name=f"gk{pr}")
                      for pr in range(2)]
                for ch in range(NCHUNK):
                    for pr in range(2):
                        o = pr * 192
                        # stationary q_pair; moving [k_pair | q_pair]
                        nc.tensor.matmul(
                            sg[pr][:, :],
                            lhsT=qkt_t[:, ch, o + 96 : o + 192],
                            rhs=qkt_t[:, ch, o : o + 192],
                            start=(ch == 0),
                            stop=(ch == NCHUNK - 1),
                        )
                        nc.tensor.matmul(
                            gk[pr][:, :],
                            lhsT=qkt_t[:, ch, o : o + 96],
                            rhs=qkt_t[:, ch, o : o + 96],
                            start=(ch == 0),
                            stop=(ch == NCHUNK - 1),
                        )
                for pr in range(2):
                    dst = s_acc[:, pr * 96 : (pr + 1) * 96]
                    if blk == 0:
                        nc.vector.tensor_copy(dst, sg[pr][:, 0:96])
                    else:
                        nc.vector.tensor_add(dst, dst, sg[pr][:, 0:96])
                    # gram diagonals via identity-mask + row-reduce
                    cq = (pr * 2) * NBLK + blk
                    ck = (pr * 2 + 1) * NBLK + blk
                    scr = scrp.tile([96, 96], FP32, tag="scr")
                    nc.vector.tensor_mul(scr[:, :], sg[pr][:, 96:192], ident96[:, :])
                    nc.vector.tensor_reduce(
                        ssq_parts[:, cq : cq + 1], scr[:, :],
                        axis=AX.X, op=ALU.add,
                    )
                    scr2 = scrp.tile([96, 96], FP32, tag="scr")
                    nc.vector.tensor_mul(scr2[:, :], gk[pr][:, :], ident96[:, :])
                    nc.vector.tensor_reduce(
                        ssq_parts[:, ck : ck + 1], scr2[:, :],
                        axis=AX.X, op=ALU.add,
                    )

            # ================= phase 2: attention finalize =================
            # cols of ssqt: 0=qA 1=kA 2=qB 3=kB
            ssqt = smallp.tile([96, 4], FP32, tag="ssqt")
            for i in range(4):
                nc.vector.tensor_reduce(
                    ssqt[:, i : i + 1],
                    ssq_parts[:, i * NBLK : (i + 1) * NBLK],
                    axis=AX.X,
                    op=ALU.add,
                )
            rec = smallp.tile([96, 4], FP32, tag="rec")
            nc.vector.reciprocal(rec[:, :], ssqt[:, :])
            rn = smallp.tile([96, 4], FP32, tag="rn")
            nc.scalar.activation(rn[:, :], rec[:, :], AF.Sqrt)
            # fold temperature into the q-side scales (tempc: ones in k cols)
            nc.vector.tensor_mul(rn[:, :], rn[:, :], tc_sb[:, :])

            # per-column -> row vectors: plain K=96 matmul against identity
            srow = smallp.tile([1, 4 * 96], FP32, tag="srow")
            for i in range(4):
                tp = sps.tile([1, 96], FP32, tag="g0", name="tp")
                nc.tensor.matmul(
                    tp[:, :], lhsT=rn[:, i : i + 1], rhs=ident96[:, :],
                    start=True, stop=True,
                )
                nc.vector.tensor_copy(srow[0:1, i * 96 : (i + 1) * 96], tp[:, :])

            # scale S, masked softmax per pair, assemble blockdiag attn
            attn_bd = smallp.tile([96, 2, CLOC], FP32, tag="abd")
            nc.vector.memset(attn_bd[:], 0.0)
            for pr in range(2):
                op = sps.tile([96, 96], FP32, tag="g0")
                nc.tensor.matmul(
                    op[:, :],
                    lhsT=srow[0:1, (2 * pr) * 96 : (2 * pr + 1) * 96],
                    rhs=srow[0:1, (2 * pr + 1) * 96 : (2 * pr + 2) * 96],
                    start=True,
                    stop=True,
                )
                spr = s_acc[:, pr * 96 : (pr + 1) * 96]
                nc.vector.tensor_mul(spr, spr, op[:, :])
                nc.vector.tensor_add(spr, spr, mask_sb[:, :])
                nmax = smallp.tile([96, 1], FP32, tag="nmax")
                nc.vector.tensor_reduce(
                    nmax[:, :], spr, axis=AX.X, op=ALU.max, negate=True
                )
                esum = smallp.tile([96, 1], FP32, tag="esum")
                dst = attn_bd[:, pr, pr * 96 : (pr + 1) * 96]
                nc.scalar.activation(
                    dst, spr, AF.Exp, bias=nmax[:, :], accum_out=esum[:, :]
                )
                rcp = smallp.tile([96, 1], FP32, tag="rcp")
                nc.vector.reciprocal(rcp[:, :], esum[:, :])
                nc.vector.tensor_scalar_mul(dst, dst, rcp[:, :])

            # M2.T = blockdiag(attn).T @ Wp_local.T   [192, 384], pair chunks
            m2t = smallp.tile([96, 2, C], DT, tag="m2t")
            for pr in range(2):
                ps = mmps.tile([128, 512], FP32, tag="mm")
                for cc in range(2):
                    nc.tensor.matmul(
                        ps[0:96, 0:C],
                        lhsT=attn_bd[:, cc, pr * 96 : (pr + 1) * 96],
                        rhs=wp_sb[:, cc, :],
                        start=(cc == 0),
                        stop=(cc == 1),
                    )
                nc.vector.tensor_copy(m2t[:, pr, :], ps[0:96, 0:C])

            # M3.T = Wv_local.T @ M2.T  [384, 384] = ((proj @ attn_bd) @ Wv).T
            m3t = smallp.tile([128, 3, C], DT, tag="m3t")
            for mt in range(3):
                ps = mmps.tile([128, 512], FP32, tag="mm")
                for cc in range(2):
                    nc.tensor.matmul(
                        ps[:, 0:C],
                        lhsT=wv_sb[:, cc, mt * 128 : (mt + 1) * 128],
                        rhs=m2t[:, cc, :],
                        start=(cc == 0),
                        stop=(cc == 1),
                    )
                nc.vector.tensor_copy(m3t[:, mt, :], ps[:, 0:C])

            # exchange M3 with the pair core and sum: each core then owns
            # out = (M3_even + M3_odd) @ x for its half of n (the host gives
            # odd cores a half-rotated x, so "its half" is always cols 0:N/2)
            m3_ib = dramp.tile([128, 3 * C], DT, tag="m3ib")
            m3_ob = dramp.tile([128, 3 * C], DT, tag="m3ob")
            nc.gpsimd.dma_start(m3_ib[:, :], m3t[:, :, :])
            nc.gpsimd.collective_compute(
                "AllReduce",
                ALU.add,
                replica_groups=[[0, 1], [2, 3], [4, 5], [6, 7]],
                ins=[m3_ib.opt()],
                outs=[m3_ob.opt()],
            )
            m3s = smallp.tile([128, 3, C], DT, tag="m3s")
            nc.gpsimd.dma_start(m3s[:, :, :], m3_ob[:, :])

            # ============= phase 3: out = M3sum @ x (resident) =============
            for blk in range(NBLK // 2):
                n0 = blk * NB
                for mt in range(3):
                    pss = [mmps.tile([128, 512], FP32, tag="mm", name=f"po{i}")
                           for i in range(N512)]
                    for kc in range(3):
                        for n5 in range(N512):
                            nc.tensor.matmul(
                                pss[n5][:, :],
                                lhsT=m3s[:, kc, mt * 128 : (mt + 1) * 128],
                                rhs=x_res[:, kc, n0 + n5 * 512 : n0 + (n5 + 1) * 512],
                                start=(kc == 0),
                                stop=(kc == 2),
                            )
                    for n5 in range(N512):
                        ot = opool.tile([128, 512], FP32, tag="ost")
                        if n5 % 3 == 0:
                            nc.scalar.copy(ot[:, :], pss[n5][:, :])
                        else:
                            nc.vector.tensor_copy(ot[:, :], pss[n5][:, :])
                        nc.sync.dma_start(
                            o_v[mt, :, n0 + n5 * 512 : n0 + (n5 + 1) * 512],
                            ot[:, :],
                        )
    nc.compile()
    return nc


_NC = None


def _get_nc():
    global _NC
    if _NC is None:
        _NC = build_nc()
    return _NC


def make_in_maps(x, qkv_w, proj_w, temperature):
    x2 = np.ascontiguousarray(np.asarray(x).reshape(B, C, N).astype(NPDT))
    x2r = np.ascontiguousarray(
        np.concatenate([x2[:, :, N // 2 :], x2[:, :, : N // 2]], axis=2)
    )
    qkv_w = np.asarray(qkv_w, np.float32)
    proj_w = np.asarray(proj_w, np.float32)
    tvals = np.asarray(temperature, np.float32).reshape(HEADS)
    in_maps = []
    for core in range(8):
        bi, hg = core // 2, core % 2
        heads = list(range(hg * HLOC, (hg + 1) * HLOC))
        # qkv rows in wqkvT column order [kA | qA | kB | qB]
        rows = []
        for pr in range(2):
            ph = heads[2 * pr : 2 * pr + 2]
            rows += [C + hh * CH + j for hh in ph for j in range(CH)]      # k
            rows += [hh * CH + j for hh in ph for j in range(CH)]          # q
        wqkvT = np.ascontiguousarray(qkv_w[rows, :].T.astype(NPDT))  # [384, 384]
        vrows = [2 * C + hh * CH + j for hh in heads for j in range(CH)]
        wvm = np.ascontiguousarray(qkv_w[vrows, :].astype(NPDT))     # [192, 384]
        cols = [hh * CH + j for hh in heads for j in range(CH)]
        wprojT = np.ascontiguousarray(proj_w[:, cols].T)             # [192, 384]
        # tempc cols: 0=qA 1=kA 2=qB 3=kB; temperature applies to q side
        tcol = np.ones((96, 4), np.float32)
        for pr in range(2):
            ph = heads[2 * pr : 2 * pr + 2]
            tcol[0:48, 2 * pr] = tvals[ph[0]]
            tcol[48:96, 2 * pr] = tvals[ph[1]]
        mask = np.full((96, 96), -1e30, np.float32)
        mask[:48, :48] = 0.0
        mask[48:, 48:] = 0.0
        in_maps.append(
            {"x_in": (x2 if hg == 0 else x2r)[bi],
             "wqkvT": wqkvT, "wprojT": wprojT, "wv": wvm,
             "tempc": tcol, "bdmask": mask}
        )
    return in_maps


def kernel(x, qkv_w, proj_w, temperature, **run_kwargs):
    nc = _get_nc()
    in_maps = make_in_maps(x, qkv_w, proj_w, temperature)
    res = bass_utils.run_bass_kernel_spmd(
        nc, in_maps, core_ids=list(range(8)), **run_kwargs
    )
    outs = [r["outp"] for r in res.results]
    out = np.empty((B, C, N), np.float32)
    for bi in range(B):
        out[bi, :, : N // 2] = outs[2 * bi]
        out[bi, :, N // 2 :] = outs[2 * bi + 1]
    out = out.reshape(B, C, H, W)
    kernel.last_results = res
    return out


# revision 32
# speedup vs baseline: 1.4234x; 1.0099x over previous
"""TRN2 Bass/Tile kernel for nn_CAttention (b=4, c=384, hw=128x128, 8 heads).

Math (per batch b, x flattened to [384, n], n = 16384):
  qkv = qkv_w @ x; q,k l2-normalized along n;
  attn = softmax((q_n @ k_n^T) * temp) per head [48,48]
  out = proj_w @ concat_h(attn_h @ v_h)

Sharding: 8 cores = (batch 0..3) x (head-half 0..1); host sums the two
head-half partial projections per batch.

Per-core pipeline (4 local heads = 2 "pairs" of 2 heads):
  phase 1 (stream n in 2048-blocks, x parked SBUF-resident as fp16):
    q,k produced directly TRANSPOSED ([n, 384] layout) via x-stationary
    matmuls.  Raw scores S = q_raw @ k_raw^T accumulate in PSUM per pair;
    the same matmul's moving operand carries [k_pair | q_pair] so its
    columns 96:192 give the q Gram block (diag = sum-of-squares for the
    l2 norm); a second small matmul gives the k Gram.  Normalization is
    applied post-hoc: S <- S * outer(1/||q_c||, 1/||k_d||) * temp.
  phase 2: Gram diagonals -> 1/sqrt -> row vectors (tiny identity
    matmuls), outer-product scaling, masked softmax per pair ([96,96]
    with -1e30 off-block additive mask), blockdiag attn assembly.
    M2.T = blockdiag(attn).T @ Wp_local.T;  M3.T = Wv_local.T @ M2.T --
    attn@v and the projection fold into a single [384,384] matrix M3.
  phase 3: out_partial = M3 @ x from resident x, streamed to DRAM.

Data tiles fp16 (PE full rate, fp32 PSUM accumulation); end-to-end error
vs the fp32 reference ~4e-4.

Hardware notes: tensor_tensor_reduce reading PSUM crashes the exec unit;
gram diagonals use tensor_mul + tensor_reduce instead.  GPSIMD C-axis
reduce is element-serial (~1 elem/ns) - never use it for bulk data.
"""

import numpy as np

import concourse.bass as bass
import concourse.mybir as mybir
import concourse.tile as tile
from concourse import bacc, bass_utils
from concourse.masks import make_identity

FP32 = mybir.dt.float32
FP16 = mybir.dt.float16
AF = mybir.ActivationFunctionType
ALU = mybir.AluOpType
AX = mybir.AxisListType

# problem dims
B = 4
C = 384
H = W = 128
N = H * W          # 16384
HEADS = 8
CH = 48            # per-head channels
HLOC = 4           # heads per core
CLOC = HLOC * CH   # 192

DT = FP16
NPDT = np.float16

# tiling
NB = 2048          # n-block for the streaming phase
NBLK = N // NB     # 8
NCHUNK = NB // 128 # 16 chunks / block
N512 = NB // 512   # 4


def build_nc():
    nc = bacc.Bacc("TRN2", target_bir_lowering=False, debug=False, num_devices=8)

    # wqkvT columns: [kA(96) | qA(96) | kB(96) | qB(96)]
    x_in = nc.dram_tensor("x_in", [C, N], DT, kind="ExternalInput").ap()
    wqkvT = nc.dram_tensor("wqkvT", [C, 2 * CLOC], DT, kind="ExternalInput").ap()
    wprojT = nc.dram_tensor("wprojT", [CLOC, C], FP32, kind="ExternalInput").ap()
    wv = nc.dram_tensor("wv", [CLOC, C], DT, kind="ExternalInput").ap()
    tempc = nc.dram_tensor("tempc", [96, 4], FP32, kind="ExternalInput").ap()
    bdmask = nc.dram_tensor("bdmask", [96, 96], FP32, kind="ExternalInput").ap()
    outp = nc.dram_tensor("outp", [C, N // 2], FP32, kind="ExternalOutput").ap()

    x_v = x_in.rearrange("(t p) n -> t p n", p=128)    # [3, 128, N]
    w_v = wqkvT.rearrange("(t p) m -> t p m", p=128)   # [3, 128, 384]
    wp_v = wprojT.rearrange("(s p) m -> s p m", p=96)  # [2, 96, 384]
    wv_v = wv.rearrange("(s p) m -> s p m", p=96)      # [2, 96, 384]
    o_v = outp.rearrange("(t p) n -> t p n", p=128)    # [3, 128, N/2]

    with tile.TileContext(nc) as tc:
        with (
            tc.tile_pool(name="const", bufs=1) as constp,
            tc.tile_pool(name="xres", bufs=1) as xpool,
            tc.tile_pool(name="qkt", bufs=2) as qktpool,
            tc.tile_pool(name="small", bufs=1) as smallp,
            tc.tile_pool(name="scr", bufs=2) as scrp,
            tc.tile_pool(name="ostage", bufs=3) as opool,
            tc.tile_pool(name="mm_psum", bufs=4, space="PSUM") as mmps,
            tc.tile_pool(name="s_psum", bufs=1, space="PSUM") as sps,
            tc.tile_pool(name="dram", bufs=1, space="DRAM") as dramp,
        ):
            # ---- constants / weights ----
            ident96 = constp.tile([96, 96], FP32, tag="ident96")
            make_identity(nc, ident96[:])

            w_sb = constp.tile([128, 3, 2 * CLOC], DT, tag="wqkv")
            for kc in range(3):
                nc.sync.dma_start(w_sb[:, kc, :], w_v[kc])
            wp_sb = constp.tile([96, 2, C], FP32, tag="wproj")
            for cc in range(2):
                nc.sync.dma_start(wp_sb[:, cc, :], wp_v[cc])
            wv_sb = constp.tile([96, 2, C], DT, tag="wv")
            for cc in range(2):
                nc.sync.dma_start(wv_sb[:, cc, :], wv_v[cc])
            tc_sb = constp.tile([96, 4], FP32, tag="tempc")
            nc.sync.dma_start(tc_sb[:], tempc[:])
            mask_sb = constp.tile([96, 96], FP32, tag="mask")
            nc.sync.dma_start(mask_sb[:], bdmask[:])

            # ---- persistent state ----
            x_res = xpool.tile([128, 3, N], DT, tag="xres")       # resident input
            s_acc = smallp.tile([96, 2 * 96], FP32, tag="sacc")   # S_raw per pair
            # gram-diag partials: col = (pr*2 + {0:q,1:k}) * NBLK + blk
            ssq_parts = smallp.tile([96, 4 * NBLK], FP32, tag="ssqp")

            # ================= phase 1: stream over n =================
            for blk in range(NBLK):
                n0 = blk * NB
                for ct in range(3):
                    nc.sync.dma_start(
                        x_res[:, ct, n0 : n0 + NB], x_v[ct, :, n0 : n0 + NB]
                    )

                # q,k transposed: [n-chunk, 384] = x_chunk.T @ Wqk.T
                qkt_t = qktpool.tile([128, NCHUNK, 2 * CLOC], DT, tag="qkt")
                for ch in range(NCHUNK):
                    ps = mmps.tile([128, 512], FP32, tag="mm")
                    for kc in range(3):
                        nc.tensor.matmul(
                            ps[:, 0 : 2 * CLOC],
                            lhsT=x_res[:, kc, n0 + ch * 128 : n0 + (ch + 1) * 128],
                            rhs=w_sb[:, kc, :],
                            start=(kc == 0),
                            stop=(kc == 2),
                        )
                    if ch % 2 == 0:
                        nc.scalar.copy(qkt_t[:, ch, :], ps[:, 0 : 2 * CLOC])
                    else:
                        nc.vector.tensor_copy(qkt_t[:, ch, :], ps[:, 0 : 2 * CLOC])

                # scores + gram blocks, accumulated over chunks in PSUM
                sg = [sps.tile([96, 192], FP32, tag=f"s{pr}", name=f"sg{pr}")
                      for pr in range(2)]
                gk = [sps.tile([96, 96], FP32, tag=f"g{pr}", name=f"gk{pr}")
                      for pr in range(2)]
                for ch in range(NCHUNK):
                    for pr in range(2):
                        o = pr * 192
                        # stationary q_pair; moving [k_pair | q_pair]
                        nc.tensor.matmul(
                            sg[pr][:, :],
                            lhsT=qkt_t[:, ch, o + 96 : o + 192],
                            rhs=qkt_t[:, ch, o : o + 192],
                            start=(ch == 0),
                            stop=(ch == NCHUNK - 1),
                        )
                        nc.tensor.matmul(
                            gk[pr][:, :],
                            lhsT=qkt_t[:, ch, o : o + 96],
                            rhs=qkt_t[:, ch, o : o + 96],
                            start=(ch == 0),
                            stop=(ch == NCHUNK - 1),
                        )
                for pr in range(2):
                    dst = s_acc[:, pr * 96 : (pr + 1) * 96]
                    if blk == 0:
                        nc.vector.tensor_copy(dst, sg[pr][:, 0:96])
                    else:
                        nc.vector.tensor_add(dst, dst, sg[pr][:, 0:96])
                    # gram diagonals via identity-mask + row-reduce
                    cq = (pr * 2) * NBLK + blk
                    ck = (pr * 2 + 1) * NBLK + blk
                    scr = scrp.tile([96, 96], FP32, tag="scr")
                    nc.vector.tensor_mul(scr[:, :], sg[pr][:, 96:192], ident96[:, :])
                    nc.vector.tensor_reduce(
                        ssq_parts[:, cq : cq + 1], scr[:, :],
                        axis=AX.X, op=ALU.add,
                    )
                    scr2 = scrp.tile([96, 96], FP32, tag="scr")
                    nc.vector.tensor_mul(scr2[:, :], gk[pr][:, :], ident96[:, :])
                    nc.vector.tensor_reduce(
                        ssq_parts[:, ck : ck + 1], scr2[:, :],
                        axis=AX.X, op=ALU.add,
                    )

            # ================= phase 2: attention finalize =================
            # cols of ssqt: 0=qA 1=kA 2=qB 3=kB
            ssqt = smallp.tile([96, 4], FP32, tag="ssqt")
            for i in range(4):
                nc.vector.tensor_reduce(
                    ssqt[:, i : i + 1],
                    ssq_parts[:, i * NBLK : (i + 1) * NBLK],
                    axis=AX.X,
                    op=ALU.add,
                )
            rec = smallp.tile([96, 4], FP32, tag="rec")
            nc.vector.reciprocal(rec[:, :], ssqt[:, :])
            rn = smallp.tile([96, 4], FP32, tag="rn")
            nc.scalar.activation(rn[:, :], rec[:, :], AF.Sqrt)
            # fold temperature into the q-side scales (tempc: ones in k cols)
            nc.vector.tensor_mul(rn[:, :], rn[:, :], tc_sb[:, :])

            # per-column -> row vectors: plain K=96 matmul against identity
            srow = smallp.tile([1, 4 * 96], FP32, tag="srow")
            for i in range(4):
                tp = sps.tile([1, 96], FP32, tag="g0", name="tp")
                nc.tensor.matmul(
                    tp[:, :], lhsT=rn[:, i : i + 1], rhs=ident96[:, :],
                    start=True, stop=True,
                )
                nc.vector.tensor_copy(srow[0:1, i * 96 : (i + 1) * 96], tp[:, :])

            # scale S, masked softmax per pair, assemble blockdiag attn
            attn_bd = smallp.tile([96, 2, CLOC], FP32, tag="abd")
            nc.vector.memset(attn_bd[:], 0.0)
            for pr in range(2):
                op = sps.tile([96, 96], FP32, tag="g0")
                nc.tensor.matmul(
                    op[:, :],
                    lhsT=srow[0:1, (2 * pr) * 96 : (2 * pr + 1) * 96],
                    rhs=srow[0:1, (2 * pr + 1) * 96 : (2 * pr + 2) * 96],
                    start=True,
                    stop=True,
                )
                spr = s_acc[:, pr * 96 : (pr + 1) * 96]
                nc.vector.tensor_mul(spr, spr, op[:, :])
                nc.vector.tensor_add(spr, spr, mask_sb[:, :])
                nmax = smallp.tile([96, 1], FP32, tag="nmax")
                nc.vector.tensor_reduce(
                    nmax[:, :], spr, axis=AX.X, op=ALU.max, negate=True
                )
                esum = smallp.tile([96, 1], FP32, tag="esum")
                dst = attn_bd[:, pr, pr * 96 : (pr + 1) * 96]
                nc.scalar.activation(
                    dst, spr, AF.Exp, bias=nmax[:, :], accum_out=esum[:, :]
                )
                rcp = smallp.tile([96, 1], FP32, tag="rcp")
                nc.vector.reciprocal(rcp[:, :], esum[:, :])
                nc.vector.tensor_scalar_mul(dst, dst, rcp[:, :])

            # M2.T = blockdiag(attn).T @ Wp_local.T   [192, 384], pair chunks
            m2t = smallp.tile([96, 2, C], DT, tag="m2t")
            for pr in range(2):
                ps = mmps.tile([128, 512], FP32, tag="mm")
                for cc in range(2):
                    nc.tensor.matmul(
                        ps[0:96, 0:C],
                        lhsT=attn_bd[:, cc, pr * 96 : (pr + 1) * 96],
                        rhs=wp_sb[:, cc, :],
                        start=(cc == 0),
                        stop=(cc == 1),
                    )
                nc.vector.tensor_copy(m2t[:, pr, :], ps[0:96, 0:C])

            # M3.T = Wv_local.T @ M2.T  [384, 384] = ((proj @ attn_bd) @ Wv).T
            m3t = smallp.tile([128, 3, C], DT, tag="m3t")
            for mt in range(3):
                ps = mmps.tile([128, 512], FP32, tag="mm")
                for cc in range(2):
                    nc.tensor.matmul(
                        ps[:, 0:C],
                        lhsT=wv_sb[:, cc, mt * 128 : (mt + 1) * 128],
                        rhs=m2t[:, cc, :],
                        start=(cc == 0),
                        stop=(cc == 1),
                    )
                nc.vector.tensor_copy(m3t[:, mt, :], ps[:, 0:C])

            # exchange M3 with the pair core and sum: each core then owns
            # out = (M3_even + M3_odd) @ x for its half of n (the host gives
            # odd cores a half-rotated x, so "its half" is always cols 0:N/2)
            m3_ib = dramp.tile([128, 3 * C], DT, tag="m3ib")
            m3_ob = dramp.tile([128, 3 * C], DT, tag="m3ob")
            nc.gpsimd.dma_start(m3_ib[:, :], m3t[:, :, :])
            nc.gpsimd.collective_compute(
                "AllReduce",
                ALU.add,
                replica_groups=[[0, 1], [2, 3], [4, 5], [6, 7]],
                ins=[m3_ib.opt()],
                outs=[m3_ob.opt()],
            )
            m3s = smallp.tile([128, 3, C], DT, tag="m3s")
            nc.gpsimd.dma_start(m3s[:, :, :], m3_ob[:, :])

            # ============= phase 3: out = M3sum @ x (resident) =============
            for blk in range(NBLK // 2):
                n0 = blk * NB
                for mt in range(3):
                    pss = [mmps.tile([128, 512], FP32, tag="mm", name=f"po{i}")
                           for i in range(N512)]
                    for kc in range(3):
                        for n5 in range(N512):
                            nc.tensor.matmul(
                                pss[n5][:, :],
                                lhsT=m3s[:, kc, mt * 128 : (mt + 1) * 128],
                                rhs=x_res[:, kc, n0 + n5 * 512 : n0 + (n5 + 1) * 512],
                                start=(kc == 0),
                                stop=(kc == 2),
                            )
                    for n5 in range(N512):
                        ot = opool.tile([128, 512], FP32, tag="ost")
                        if n5 % 3 == 0:
                            nc.scalar.copy(ot[:, :], pss[n5][:, :])
                        else:
                            nc.vector.tensor_copy(ot[:, :], pss[n5][:, :])
                        nc.sync.dma_start(
                            o_v[mt, :, n0 + n5 * 512 : n0 + (n5 + 1) * 512],
                            ot[:, :],
                        )
    nc.compile()
    return nc


_NC = None


def _get_nc():
    global _NC
    if _NC is None:
        _NC = build_nc()
    return _NC


def make_in_maps(x, qkv_w, proj_w, temperature):
    x2 = np.ascontiguousarray(np.asarray(x).reshape(B, C, N).astype(NPDT))
    x2r = np.ascontiguousarray(
        np.concatenate([x2[:, :, N // 2 :], x2[:, :, : N // 2]], axis=2)
    )
    qkv_w = np.asarray(qkv_w, np.float32)
    proj_w = np.asarray(proj_w, np.float32)
    tvals = np.asarray(temperature, np.float32).reshape(HEADS)
    in_maps = []
    for core in range(8):
        bi, hg = core // 2, core % 2
        heads = list(range(hg * HLOC, (hg + 1) * HLOC))
        # qkv rows in wqkvT column order [kA | qA | kB | qB]
        rows = []
        for pr in range(2):
            ph = heads[2 * pr : 2 * pr + 2]
            rows += [C + hh * CH + j for hh in ph for j in range(CH)]      # k
            rows += [hh * CH + j for hh in ph for j in range(CH)]          # q
        wqkvT = np.ascontiguousarray(qkv_w[rows, :].T.astype(NPDT))  # [384, 384]
        vrows = [2 * C + hh * CH + j for hh in heads for j in range(CH)]
        wvm = np.ascontiguousarray(qkv_w[vrows, :].astype(NPDT))     # [192, 384]
        cols = [hh * CH + j for hh in heads for j in range(CH)]
        wprojT = np.ascontiguousarray(proj_w[:, cols].T)             # [192, 384]
        # tempc cols: 0=qA 1=kA 2=qB 3=kB; temperature applies to q side
        tcol = np.ones((96, 4), np.float32)
        for pr in range(2):
            ph = heads[2 * pr : 2 * pr + 2]
            tcol[0:48, 2 * pr] = tvals[ph[0]]
            tcol[48:96, 2 * pr] = tvals[ph[1]]
        mask = np.full((96, 96), -1e30, np.float32)
        mask[:48, :48] = 0.0
        mask[48:, 48:] = 0.0
        in_maps.append(
            {"x_in": (x2 if hg == 0 else x2r)[bi],
             "wqkvT": wqkvT, "wprojT": wprojT, "wv": wvm,
             "tempc": tcol, "bdmask": mask}
        )
    return in_maps


def _assemble(outs):
    out = np.empty((B, C, N), np.float32)
    for bi in range(B):
        out[bi, :, : N // 2] = outs[2 * bi]
        out[bi, :, N // 2 :] = outs[2 * bi + 1]
    return out.reshape(B, C, H, W)


def _run_direct(x, qkv_w, proj_w, temperature, **run_kwargs):
    nc = _get_nc()
    in_maps = make_in_maps(x, qkv_w, proj_w, temperature)
    res = bass_utils.run_bass_kernel_spmd(
        nc, in_maps, core_ids=list(range(8)), **run_kwargs
    )
    kernel.last_results = res
    return _assemble([r["outp"] for r in res.results])


def _run_subprocess(x, qkv_w, proj_w, temperature):
    # fresh interpreter -> fresh PJRT client; recovers from a wedged device
    import os
    import subprocess
    import sys
    import tempfile

    with tempfile.TemporaryDirectory() as td:
        ipath = os.path.join(td, "in.npz")
        opath = os.path.join(td, "out.npy")
        np.savez(ipath, x=x, qkv_w=qkv_w, proj_w=proj_w,
                 temperature=temperature)
        code = (
            "import sys, numpy as np\n"
            f"sys.path.insert(0, {os.path.dirname(os.path.abspath(__file__))!r})\n"
            "import kernel as K\n"
            f"d = np.load({ipath!r})\n"
            "out = K._run_direct(d['x'], d['qkv_w'], d['proj_w'],"
            " d['temperature'])\n"
            f"np.save({opath!r}, out)\n"
        )
        subprocess.run([sys.executable, "-c", code], check=True, timeout=1200)
        return np.load(opath)


def kernel(x, qkv_w, proj_w, temperature, **run_kwargs):
    try:
        return _run_direct(x, qkv_w, proj_w, temperature, **run_kwargs)
    except Exception:
        pass
    err = None
    for _ in range(2):
        try:
            return _run_subprocess(x, qkv_w, proj_w, temperature)
        except Exception as e:  # noqa: PERF203
            err = e
    raise err
